# revision 65
# baseline (speedup 1.0000x reference)
"""AttentionPairBias kernel for 8 Trainium2 NeuronCores.

Sharding: rows of the query sequence (S=1024) are split across the 8 cores
(128 rows each). The pair tensor z's bias contribution, the softmax and the
output rows are all embarrassingly parallel in the query dimension, so no
collectives are needed; each core reads its own 128x1024x128 slice of z.

Per-core pipeline:
  1. z arrives from the host pre-transposed ([c, t] per query row) and cast
     to fp8e4m3, so group loads are plain contiguous DMAs.
  2. The pair-bias linear runs as fp8 DoubleRow matmuls (K=256): the two
     k-tiles are the two t-halves of a row's zT tile, separated by
     block-diagonal 64-wide weights, so each 512-col stream covers all
     1024 t positions. Outputs pack 2 query rows per PSUM bank at tile
     positions (0,0)/(0,64). Weight columns also carry the LayerNorm mean
     (w=32/128) and, in a second accumulating pass over z^2, E[z^2] (w=8),
     with a global x32 scale that cancels inside rsqrt. The per-head mean
     correction c1[h]/128 is folded into the weights, so bias = r * y
     directly. The ln_b term is dropped (constant along t, softmax-
     invariant).
  3. y rows round-trip through DRAM to re-slice [head-per-partition] into
     [row-per-partition] tiles; stats (mu, E[z^2]) are prefetched back
     per-group so the variance math never waits on a bulk readback.
  4. Per head: scores = qk/sqrt(hd) + r*y_h -> PE transpose -> exp on ACT
     (max-subtraction-free: |scores| < 4) -> A@[V|1] gives o and the
     softmax denominator in one accumulation chain.
  5. sigmoid gate, output projection.
"""

import os
import sys
import types
import numpy as np

for _p in ("/opt/trn_rl_repo", "/root/.axon_site/_ro/trn_rl_repo"):
    if os.path.isdir(_p) and _p not in sys.path:
        sys.path.append(_p)

import ml_dtypes
from contextlib import ExitStack

import concourse.bass as bass
import concourse.mybir as mybir
import concourse.tile as tile
from concourse import bacc
from concourse.bass import ds, ts
from concourse.masks import make_identity

BF16 = mybir.dt.bfloat16
FP32 = mybir.dt.float32
FP8 = mybir.dt.float8e4
AF = mybir.ActivationFunctionType
ALU = mybir.AluOpType
DR = mybir.MatmulPerfMode.DoubleRow

S = 1024
D = 768
H = 16
HD = 48
HDP = 64            # padded head dim (2 heads per 128-partition block)
DP = H * HDP        # 1024
DZ = 128
EPS = 1e-5
N_CORES = 8
RPC = S // N_CORES  # 128 rows per core
ISQ = float(HD) ** -0.5
ZSC = 32.0          # fp8 weight pre-scale; cancels inside rsqrt

_CACHE = {}


def _build():
    nc = bacc.Bacc("TRN2", target_bir_lowering=False, debug=False,
                   num_devices=N_CORES)

    zb = nc.dram_tensor("zb", [RPC // 4, DZ, 4, S], FP8,
                        kind="ExternalInput").ap()
    sT = nc.dram_tensor("sT", [D, S], BF16, kind="ExternalInput").ap()
    sTc = nc.dram_tensor("sTc", [D, RPC], BF16, kind="ExternalInput").ap()
    WqT = nc.dram_tensor("WqT", [D, DP], BF16, kind="ExternalInput").ap()
    WkT = nc.dram_tensor("WkT", [D, DP], BF16, kind="ExternalInput").ap()
    WvT = nc.dram_tensor("WvT", [D, DP], BF16, kind="ExternalInput").ap()
    WgT = nc.dram_tensor("WgT", [D, D], BF16, kind="ExternalInput").ap()
    WoT = nc.dram_tensor("WoT", [D, D], BF16, kind="ExternalInput").ap()
    bqs = nc.dram_tensor("bqs", [DP], FP32, kind="ExternalInput").ap()
    Wy8 = nc.dram_tensor("Wy8", [2, DZ, 2, 128], FP8, kind="ExternalInput").ap()
    Ws8 = nc.dram_tensor("Ws8", [2, DZ, 2, 128], FP8, kind="ExternalInput").ap()
    out = nc.dram_tensor("out", [RPC, D], FP32, kind="ExternalOutput").ap()

    with tile.TileContext(nc) as tc, ExitStack() as ctx:
        consts = ctx.enter_context(tc.tile_pool(name="consts", bufs=1))
        dram = ctx.enter_context(tc.tile_pool(name="dram", bufs=1, space="DRAM"))

        # sT on sync, wk on scalar: the two 1.5MB tensors feeding the first
        # PE matmuls load in parallel (~135 GB/s per queue). Small loads go
        # first on scalar: the framework's conservative startup semaphore
        # makes early matmuls wait on whole dma batches.
        # tiny pair-bias weights lead the sync queue so the first DR matmuls
        # never wait behind the big projection weights (coarse dma batching)
        wy_sb = consts.tile([128, 2, 2, 128], FP8, name="wy_sb")
        nc.sync.dma_start(wy_sb[:], Wy8.rearrange("r c i m -> c r i m"))
        ws_sb = consts.tile([128, 2, 2, 128], FP8, name="ws_sb")
        nc.sync.dma_start(ws_sb[:], Ws8.rearrange("r c i m -> c r i m"))
        bq_sb = consts.tile([128, 8], FP32, name="bq_sb")
        nc.sync.dma_start(bq_sb[:], bqs.rearrange("(b p) -> p b", p=128))
        sT_sb = consts.tile([128, 6, S], BF16, name="sT_sb")
        nc.sync.dma_start(sT_sb[:], sT.rearrange("(a p) n -> p a n", p=128))
        wk_sb = consts.tile([128, 6, DP], BF16, name="wk_sb")
        nc.scalar.dma_start(wk_sb[:], WkT.rearrange("(a p) n -> p a n", p=128))
        sTc_sb = consts.tile([128, 6, RPC], BF16, name="sTc_sb")
        nc.scalar.dma_start(sTc_sb[:], sTc.rearrange("(a p) n -> p a n", p=128))
        wq_sb = consts.tile([128, 6, DP], BF16, name="wq_sb")
        nc.scalar.dma_start(wq_sb[:], WqT.rearrange("(a p) n -> p a n", p=128))
        wg_sb = consts.tile([128, 6, D], BF16, name="wg_sb")
        wo_sb = consts.tile([128, 6, D], BF16, name="wo_sb")
        # gpsimd queue: z group 0 (emitted in the loop), then v weights,
        # then the z-odd/y/stats stream
        wv_sb = consts.tile([128, 6, DP], BF16, name="wv_sb")

        def emit_wv():
            nc.gpsimd.dma_start(wv_sb[:],
                                WvT.rearrange("(a p) n -> p a n", p=128))
        ident = consts.tile([128, 128], BF16, name="ident")
        make_identity(nc, ident[:])
        eps_sb = consts.tile([128, 1], FP32, name="eps_sb")
        nc.vector.memset(eps_sb[:], EPS * ZSC * ZSC)

        kT_sb = consts.tile([128, 8, S], BF16, name="kT_sb")
        v_sb = consts.tile([128, 8, H, HDP + 1], BF16, name="v_sb")
        qT_sb = consts.tile([128, 8, RPC], BF16, name="qT_sb")
        g_sb = consts.tile([128, D], BF16, name="g_sb")
        oall = consts.tile([128, D], BF16, name="oall")
        # stats: [s, hi, {mu, ez2}, j] prefetched per-group from y_dram
        stats_sb = consts.tile([128, 2, 2, 512], BF16, name="stats_sb")
        r_sb = consts.tile([128, S], BF16, name="r_sb")
        var_sb = consts.tile([128, S], BF16, name="var_sb")

        # y round-trip buffer, query-row-linear: [s, hi, out, j]; the group
        # write AP permutes (bank, row-in-bank) into place
        y_dram = dram.tile([RPC, 2, 32, 512], BF16)

        nc.vector.memset(v_sb[:, :, :, HDP:HDP + 1], 1.0)

        # ---- stage B (projections) + stage C (pair-bias) share pools and
        # are INTERLEAVED in program order: engine streams execute in-order,
        # so emitting all of B first would head-block stage C's squares and
        # matmuls behind B's weight loads (and vice versa) ----
        with tc.tile_pool(name="psA", bufs=2, space="PSUM") as psA, \
             tc.tile_pool(name="psY", bufs=3, space="PSUM") as psY, \
             tc.tile_pool(name="zwork", bufs=6) as zw, \
             tc.tile_pool(name="ypool", bufs=4) as yp:

            def emit_kT(blk, ch):
                p = psA.tile([128, 512], FP32, tag="pA")
                for ko in range(6):
                    nc.tensor.matmul(p[:], lhsT=wk_sb[:, ko, ts(blk, 128)],
                                     rhs=sT_sb[:, ko, ts(ch, 512)],
                                     start=(ko == 0), stop=(ko == 5))
                nc.vector.tensor_copy(kT_sb[:, blk, ts(ch, 512)], p[:])

            def emit_v(tb, ch):
                p = psA.tile([128, 512], FP32, tag="pA")
                for ko in range(6):
                    nc.tensor.matmul(p[:], lhsT=sT_sb[:, ko, ts(tb, 128)],
                                     rhs=wv_sb[:, ko, ts(ch, 512)],
                                     start=(ko == 0), stop=(ko == 5))
                nc.vector.tensor_copy(
                    v_sb[:, tb, ds(8 * ch, 8), 0:HDP],
                    p.rearrange("p (a b) -> p a b", a=8))

            def emit_q(blk):
                p = psA.tile([128, 512], FP32, tag="pA", name="pQ")[:, :RPC]
                for ko in range(6):
                    nc.tensor.matmul(p[:], lhsT=wq_sb[:, ko, ts(blk, 128)],
                                     rhs=sTc_sb[:, ko, :],
                                     start=(ko == 0), stop=(ko == 5))
                nc.scalar.activation(qT_sb[:, blk, :], p[:], AF.Identity,
                                     bias=bq_sb[:, blk:blk + 1], scale=ISQ)

            def emit_g(ch):
                w = 512 if ch == 0 else 256
                p = psA.tile([128, 512], FP32, tag="pA")
                for ko in range(6):
                    nc.tensor.matmul(p[:, :w], lhsT=sTc_sb[:, ko, :],
                                     rhs=wg_sb[:, ko, ds(512 * ch, w)],
                                     start=(ko == 0), stop=(ko == 5))
                nc.vector.tensor_copy(g_sb[:, ds(512 * ch, w)], p[:, :w])

            def _cap(f, *a):
                return lambda: f(*a)

            b_units = (
                [_cap(emit_kT, blk, ch) for blk in range(8) for ch in range(2)]
                + [_cap(emit_v, tb, ch) for tb in range(8) for ch in range(2)]
                + [_cap(emit_q, blk) for blk in range(8)]
                + [_cap(emit_g, ch) for ch in range(2)]
            )

            def emit_wgo():
                nc.scalar.dma_start(wg_sb[:],
                                    WgT.rearrange("(a p) n -> p a n", p=128))
                nc.scalar.dma_start(wo_sb[:],
                                    WoT.rearrange("(a p) n -> p a n", p=128))

            # ---- stage C: pair-bias pipeline over own z rows ----
            # z alternates between the HWDGE (sync) and SWDGE (gpsimd) rings;
            # host layout makes each group load fully contiguous per partition
            NB = len(b_units)
            NG = RPC // 4
            done = 0
            for grp in range(NG):
                zT4 = zw.tile([128, 4, S], FP8, tag="zT4")
                # all z on the SWDGE ring (its queue drains much faster);
                # y/stats round-trip traffic moves to sync in exchange
                nc.gpsimd.dma_start(zT4[:], zb[grp])
                if grp == 0:
                    emit_wv()
                sq4 = zw.tile([128, 4, S], FP8, tag="sq4")
                nc.vector.tensor_tensor(sq4[:, 0], zT4[:, 0], zT4[:, 0],
                                        ALU.mult)
                nc.vector.tensor_tensor(sq4[:, 1, 0:512], zT4[:, 1, 0:512],
                                        zT4[:, 1, 0:512], ALU.mult)
                nc.scalar.activation(sq4[:, 1, 512:1024], zT4[:, 1, 512:1024],
                                     AF.Square)
                nc.scalar.activation(sq4[:, 2], zT4[:, 2], AF.Square)
                nc.scalar.activation(sq4[:, 3], zT4[:, 3], AF.Square)
                y4 = yp.tile([128, 2, 512], BF16, tag="y4")
                for a in range(2):
                    # both rows of the bank go through full-width [128, 512]
                    # accumulating matmuls (DR at partition offset 64 trips a
                    # walrus ISA check); the row-odd weights are zero in
                    # partitions 0-63 and vice versa, so accumulation places
                    # each row's outputs without tile_position
                    pa = psY.tile([128, 512], FP32, tag=f"py{a}",
                                  name=f"py{a}")
                    prev = None
                    for u in range(2):
                        j = 2 * a + u
                        rz = zT4[:, j].rearrange("c (i t) -> c i t", i=2)
                        rs = sq4[:, j].rearrange("c (i t) -> c i t", i=2)
                        m1 = nc.tensor.matmul(pa[:], lhsT=wy_sb[:, u], rhs=rz,
                                              start=(u == 0), stop=False,
                                              perf_mode=DR)
                        if prev is not None:
                            tile.add_dep_helper(m1.ins, prev.ins, sync=False,
                                                reason="bank has_written order")
                        prev = nc.tensor.matmul(pa[:], lhsT=ws_sb[:, u], rhs=rs,
                                                start=False, stop=(u == 1),
                                                perf_mode=DR)
                    nc.vector.tensor_copy(y4[:, a, :], pa[:])
                # y write + stats prefetch on sync
                nc.sync.dma_start(
                    y_dram[ds(4 * grp, 4)].rearrange(
                        "(a rr) hi o j -> (rr hi o) a j", a=2),
                    y4[:])
                nc.sync.dma_start(
                    stats_sb[ds(4 * grp, 4)],
                    y_dram[ds(4 * grp, 4), :, 16:18, :])
                # interleave a slice of stage B's projection units
                if grp == 10:
                    emit_wgo()
                upto = NB * (grp + 1) // NG
                while done < upto:
                    b_units[done]()
                    done += 1

        # ---- stage D: r from prefetched stats (scale cancels in rsqrt) ----
        mu_f = stats_sb[:, :, 0, :]
        ez_f = stats_sb[:, :, 1, :]
        var3 = var_sb.rearrange("p (a j) -> p a j", a=2)
        nc.vector.tensor_tensor(var3, mu_f, mu_f, ALU.mult)
        nc.vector.tensor_tensor(var3, ez_f, var3, ALU.subtract)
        lnv = consts.tile([128, S], BF16, name="lnv")
        nc.scalar.activation(lnv[:], var_sb[:], AF.Ln, bias=eps_sb[:])
        nc.scalar.activation(r_sb[:], lnv[:], AF.Exp, scale=-0.5)

        # ---- stage E: attention per head ----
        with tc.tile_pool(name="psE", bufs=2, space="PSUM") as psE, \
             tc.tile_pool(name="head", bufs=2) as hw_pool:
            for h in range(H):
                po2, blk = 64 * (h % 2), h // 2
                y_h = hw_pool.tile([128, 2, 512], BF16, tag="yh")
                eng = nc.sync if h % 2 == 0 else nc.scalar
                eng.dma_start(y_h[:], y_dram[:, :, h, :])
                yf = y_h.rearrange("p a j -> p (a j)")
                t1 = hw_pool.tile([128, S], BF16, tag="t1")
                nc.vector.tensor_tensor(t1[:], yf, r_sb[:], ALU.mult)
                sc = hw_pool.tile([128, S], BF16, tag="sc")
                for ch in range(2):
                    pq = psE.tile([128, 512], FP32, tag="qk")
                    nc.tensor.matmul(pq[:],
                                     lhsT=qT_sb[ds(po2, 64), blk, :],
                                     rhs=kT_sb[ds(po2, 64), blk, ts(ch, 512)],
                                     start=True, stop=True)
                    nc.vector.tensor_tensor(sc[:, ts(ch, 512)], pq[:],
                                            t1[:, ts(ch, 512)], ALU.add)
                aT = hw_pool.tile([128, 8, 128], BF16, tag="aT")
                aTf = aT.rearrange("p a b -> p (a b)")
                for half in range(2):
                    pt = psE.tile([128, 512], BF16, tag="pt")
                    for jj in range(4):
                        nc.tensor.transpose(pt[:, ts(jj, 128)],
                                            sc[:, ts(4 * half + jj, 128)],
                                            ident[:])
                    nc.scalar.activation(aTf[:, ds(512 * half, 512)], pt[:],
                                         AF.Exp)
                po = psE.tile([128, HDP + 1], FP32, tag="po")
                for tb in range(8):
                    nc.tensor.matmul(po[:], lhsT=aT[:, tb, :],
                                     rhs=v_sb[:, tb, h, :],
                                     start=(tb == 0), stop=(tb == 7))
                dr = hw_pool.tile([128, 1], FP32, tag="dr")
                nc.vector.reciprocal(dr[:], po[:, HDP:HDP + 1])
                nc.vector.tensor_scalar(oall[:, ds(HD * h, HD)], po[:, 0:HD],
                                        dr[:], None, op0=ALU.mult)

            # ---- stage F: gate + output projection ----
            sig = hw_pool.tile([128, D], BF16, tag="sig")
            nc.scalar.activation(sig[:], g_sb[:], AF.Sigmoid)
            og = hw_pool.tile([128, D], BF16, tag="og")
            nc.vector.tensor_tensor(og[:], oall[:], sig[:], ALU.mult)
            ogT = hw_pool.tile([128, 6, 128], BF16, tag="ogT")
            for half, n in ((0, 4), (1, 2)):
                pt = psE.tile([128, 512], BF16, tag="pt")
                for jj in range(n):
                    nc.tensor.transpose(pt[:, ts(jj, 128)],
                                        og[:, ts(4 * half + jj, 128)], ident[:])
                nc.vector.tensor_copy(
                    ogT[:, ds(4 * half, n), :],
                    pt.rearrange("p (a b) -> p a b", a=4)[:, 0:n, :])
            out_sb = hw_pool.tile([128, D], FP32, tag="outsb")
            for ch, w in ((0, 512), (1, 256)):
                pf = psE.tile([128, 512], FP32, tag="qk")
                for ko in range(6):
                    nc.tensor.matmul(pf[:, :w], lhsT=ogT[:, ko, :],
                                     rhs=wo_sb[:, ko, ds(512 * ch, w)],
                                     start=(ko == 0), stop=(ko == 5))
                nc.vector.tensor_copy(out_sb[:, ds(512 * ch, w)], pf[:, :w])
            nc.sync.dma_start(out[:], out_sb[:])

    nc.compile()
    return nc


def _prep(inputs):
    bf = ml_dtypes.bfloat16
    f8 = ml_dtypes.float8_e4m3
    s = np.asarray(inputs["s"], np.float32)[0]
    z = np.asarray(inputs["z"], np.float32)[0]
    Wq = np.asarray(inputs["Wq"], np.float32)
    bq = np.asarray(inputs["bq"], np.float32)
    Wk = np.asarray(inputs["Wk"], np.float32)
    Wv = np.asarray(inputs["Wv"], np.float32)
    Wg = np.asarray(inputs["Wg"], np.float32)
    ln_w = np.asarray(inputs["ln_w"], np.float32)
    ln_b = np.asarray(inputs["ln_b"], np.float32)  # noqa: F841 (softmax-invariant)
    Wz = np.asarray(inputs["Wz"], np.float32)
    Wo = np.asarray(inputs["Wo"], np.float32)

    def pad_rows(W):
        Wp = np.zeros((DP, D), np.float32)
        for h in range(H):
            Wp[h * HDP:h * HDP + HD] = W[h * HD:(h + 1) * HD]
        return Wp

    sT = np.ascontiguousarray(s.T).astype(bf)
    WqTp = np.ascontiguousarray(pad_rows(Wq).T).astype(bf)
    WkTp = np.ascontiguousarray(pad_rows(Wk).T).astype(bf)
    WvTp = np.ascontiguousarray(pad_rows(Wv).T).astype(bf)
    WgT = np.ascontiguousarray(Wg.T).astype(bf)
    WoT = np.ascontiguousarray(Wo.T).astype(bf)
    bq_p = np.zeros(DP, np.float32)
    for h in range(H):
        bq_p[h * HDP:h * HDP + HD] = bq[h * HD:(h + 1) * HD]
    bq_p *= ISQ

    # pair-bias weights: mean correction folded in, x32 fp8 pre-scale.
    # Wy[u] places row-u-of-bank outputs at partitions 64u + {32hi + o}.
    Wzp = ln_w[None, :] * Wz                     # [H, DZ]
    c1 = Wzp.sum(-1)                             # [H]
    Wp8 = Wzp.T - c1[None, :] / DZ               # [DZ, H]
    Wy = np.zeros((2, DZ, 2, 128), np.float32)
    Ws = np.zeros((2, DZ, 2, 128), np.float32)
    for u in range(2):
        for hi in range(2):
            base = 64 * u + 32 * hi
            Wy[u, :, hi, base:base + H] = ZSC * Wp8
            Wy[u, :, hi, base + 16] = ZSC / DZ
            Ws[u, :, hi, base + 17] = ZSC * ZSC / DZ

    shared = {
        "sT": sT, "WqT": WqTp, "WkT": WkTp, "WvT": WvTp, "WgT": WgT,
        "WoT": WoT, "bqs": bq_p, "Wy8": Wy.astype(f8), "Ws8": Ws.astype(f8),
    }
    in_maps = []
    for ci in range(N_CORES):
        rows = slice(ci * RPC, (ci + 1) * RPC)
        m = dict(shared)
        m["zb"] = np.ascontiguousarray(
            z[rows].reshape(RPC // 4, 4, S, DZ).transpose(0, 3, 1, 2)
        ).astype(f8)
        m["sTc"] = np.ascontiguousarray(sT[:, rows])
        in_maps.append(m)
    return in_maps


def _install_ntff_hook():
    try:
        import antenv
        from trn_agent_boot.trn_boot import _ntff_profile_via_ctypes
        from concourse import bass_utils
        mod = types.ModuleType("antenv.axon_hooks")
        mod._hook = _ntff_profile_via_ctypes('/opt/axon/libaxon_pjrt.so')
        mod.set_axon_ntff_profile_hook = lambda h: setattr(mod, "_hook", h)
        mod.get_axon_ntff_profile_hook = lambda: mod._hook
        sys.modules["antenv.axon_hooks"] = mod
        antenv.axon_hooks = mod
        bass_utils.upload_artifacts = lambda tmpdir: tmpdir
    except Exception as e:  # profiling is best-effort
        print(f"ntff hook install failed: {e}", file=sys.stderr)


def run(inputs, trace=False):
    from concourse.bass_utils import run_bass_kernel_spmd
    in_maps = _prep(inputs)
    if "nc" not in _CACHE:
        _CACHE["nc"] = _build()
    nc = _CACHE["nc"]
    if trace:
        _install_ntff_hook()
    res = run_bass_kernel_spmd(nc, in_maps, core_ids=list(range(N_CORES)),
                               trace=trace)
    out = np.concatenate([res.results[i]["out"] for i in range(N_CORES)], axis=0)
    return out[None].astype(np.float32), res


def kernel(**inputs) -> np.ndarray:
    out, _ = run(inputs, trace=bool(os.environ.get("KERNEL_TRACE")))
    return out


# revision 67
# speedup vs baseline: 1.0477x; 1.0477x over previous
"""AttentionPairBias kernel for 8 Trainium2 NeuronCores.

Sharding: rows of the query sequence (S=1024) are split across the 8 cores
(128 rows each). The pair tensor z's bias contribution, the softmax and the
output rows are all embarrassingly parallel in the query dimension, so no
collectives are needed; each core reads its own 128x1024x128 slice of z.

Per-core pipeline:
  1. z arrives from the host pre-transposed ([c, t] per query row) and cast
     to fp8e4m3, so group loads are plain contiguous DMAs.
  2. The pair-bias linear runs as fp8 DoubleRow matmuls (K=256): the two
     k-tiles are the two t-halves of a row's zT tile, separated by
     block-diagonal 64-wide weights, so each 512-col stream covers all
     1024 t positions. Outputs pack 2 query rows per PSUM bank at tile
     positions (0,0)/(0,64). Weight columns also carry the LayerNorm mean
     (w=32/128) and, in a second accumulating pass over z^2, E[z^2] (w=8),
     with a global x32 scale that cancels inside rsqrt. The per-head mean
     correction c1[h]/128 is folded into the weights, so bias = r * y
     directly. The ln_b term is dropped (constant along t, softmax-
     invariant).
  3. y rows round-trip through DRAM to re-slice [head-per-partition] into
     [row-per-partition] tiles; stats (mu, E[z^2]) are prefetched back
     per-group so the variance math never waits on a bulk readback.
  4. Per head: scores = qk/sqrt(hd) + r*y_h -> PE transpose -> exp on ACT
     (max-subtraction-free: |scores| < 4) -> A@[V|1] gives o and the
     softmax denominator in one accumulation chain.
  5. sigmoid gate, output projection.
"""

import os
import sys
import types
import numpy as np

for _p in ("/opt/trn_rl_repo", "/root/.axon_site/_ro/trn_rl_repo"):
    if os.path.isdir(_p) and _p not in sys.path:
        sys.path.append(_p)

import ml_dtypes
from contextlib import ExitStack

import concourse.bass as bass
import concourse.mybir as mybir
import concourse.tile as tile
from concourse import bacc
from concourse.bass import ds, ts
from concourse.masks import make_identity

BF16 = mybir.dt.bfloat16
FP32 = mybir.dt.float32
FP8 = mybir.dt.float8e4
AF = mybir.ActivationFunctionType
ALU = mybir.AluOpType
DR = mybir.MatmulPerfMode.DoubleRow

S = 1024
D = 768
H = 16
HD = 48
HDP = 64            # padded head dim (2 heads per 128-partition block)
DP = H * HDP        # 1024
DZ = 128
EPS = 1e-5
N_CORES = 8
RPC = S // N_CORES  # 128 rows per core
ISQ = float(HD) ** -0.5
ZSC = 32.0          # fp8 weight pre-scale; cancels inside rsqrt

_CACHE = {}


def _build():
    nc = bacc.Bacc("TRN2", target_bir_lowering=False, debug=False,
                   num_devices=N_CORES)

    zb = nc.dram_tensor("zb", [RPC // 4, DZ, 4, S], FP8,
                        kind="ExternalInput").ap()
    sT = nc.dram_tensor("sT", [D, S], BF16, kind="ExternalInput").ap()
    sTc = nc.dram_tensor("sTc", [D, RPC], BF16, kind="ExternalInput").ap()
    WqT = nc.dram_tensor("WqT", [D, DP], BF16, kind="ExternalInput").ap()
    WkT = nc.dram_tensor("WkT", [D, DP], BF16, kind="ExternalInput").ap()
    WvT = nc.dram_tensor("WvT", [D, DP], BF16, kind="ExternalInput").ap()
    WgT = nc.dram_tensor("WgT", [D, D], BF16, kind="ExternalInput").ap()
    WoT = nc.dram_tensor("WoT", [D, D], BF16, kind="ExternalInput").ap()
    bqs = nc.dram_tensor("bqs", [DP], FP32, kind="ExternalInput").ap()
    Wy8 = nc.dram_tensor("Wy8", [2, DZ, 2, 128], FP8, kind="ExternalInput").ap()
    Ws8 = nc.dram_tensor("Ws8", [2, DZ, 2, 128], FP8, kind="ExternalInput").ap()
    out = nc.dram_tensor("out", [RPC, D], FP32, kind="ExternalOutput").ap()

    with tile.TileContext(nc) as tc, ExitStack() as ctx:
        consts = ctx.enter_context(tc.tile_pool(name="consts", bufs=1))
        dram = ctx.enter_context(tc.tile_pool(name="dram", bufs=1, space="DRAM"))

        # sT on sync, wk on scalar: the two 1.5MB tensors feeding the first
        # PE matmuls load in parallel (~135 GB/s per queue). Small loads go
        # first on scalar: the framework's conservative startup semaphore
        # makes early matmuls wait on whole dma batches.
        # tiny pair-bias weights lead the sync queue so the first DR matmuls
        # never wait behind the big projection weights (coarse dma batching)
        wy_sb = consts.tile([128, 2, 2, 128], FP8, name="wy_sb")
        nc.sync.dma_start(wy_sb[:], Wy8.rearrange("r c i m -> c r i m"))
        ws_sb = consts.tile([128, 2, 2, 128], FP8, name="ws_sb")
        nc.sync.dma_start(ws_sb[:], Ws8.rearrange("r c i m -> c r i m"))
        bq_sb = consts.tile([128, 8], FP32, name="bq_sb")
        nc.sync.dma_start(bq_sb[:], bqs.rearrange("(b p) -> p b", p=128))
        sT_sb = consts.tile([128, 6, S], BF16, name="sT_sb")
        nc.sync.dma_start(sT_sb[:], sT.rearrange("(a p) n -> p a n", p=128))
        wk_sb = consts.tile([128, 6, DP], BF16, name="wk_sb")
        nc.scalar.dma_start(wk_sb[:], WkT.rearrange("(a p) n -> p a n", p=128))
        sTc_sb = consts.tile([128, 6, RPC], BF16, name="sTc_sb")
        nc.scalar.dma_start(sTc_sb[:], sTc.rearrange("(a p) n -> p a n", p=128))
        wq_sb = consts.tile([128, 6, DP], BF16, name="wq_sb")
        nc.sync.dma_start(wq_sb[:], WqT.rearrange("(a p) n -> p a n", p=128))
        wg_sb = consts.tile([128, 6, D], BF16, name="wg_sb")
        wo_sb = consts.tile([128, 6, D], BF16, name="wo_sb")
        # gpsimd queue: z group 0 (emitted in the loop), then v weights,
        # then the z-odd/y/stats stream
        wv_sb = consts.tile([128, 6, DP], BF16, name="wv_sb")

        def emit_wv():
            nc.gpsimd.dma_start(wv_sb[:],
                                WvT.rearrange("(a p) n -> p a n", p=128))
        ident = consts.tile([128, 128], BF16, name="ident")
        make_identity(nc, ident[:])
        eps_sb = consts.tile([128, 1], FP32, name="eps_sb")
        nc.vector.memset(eps_sb[:], EPS * ZSC * ZSC)

        kT_sb = consts.tile([128, 8, S], BF16, name="kT_sb")
        v_sb = consts.tile([128, 8, H, HDP + 1], BF16, name="v_sb")
        qT_sb = consts.tile([128, 8, RPC], BF16, name="qT_sb")
        g_sb = consts.tile([128, D], BF16, name="g_sb")
        oall = consts.tile([128, D], BF16, name="oall")
        # stats: [s, hi, {mu, ez2}, j] prefetched per-group from y_dram
        stats_sb = consts.tile([128, 2, 2, 512], BF16, name="stats_sb")
        r_sb = consts.tile([128, S], BF16, name="r_sb")
        var_sb = consts.tile([128, S], BF16, name="var_sb")

        # y round-trip buffer, query-row-linear: [s, hi, out, j]; the group
        # write AP permutes (bank, row-in-bank) into place
        y_dram = dram.tile([RPC, 2, 32, 512], BF16)

        nc.vector.memset(v_sb[:, :, :, HDP:HDP + 1], 1.0)

        # ---- stage B (projections) + stage C (pair-bias) share pools and
        # are INTERLEAVED in program order: engine streams execute in-order,
        # so emitting all of B first would head-block stage C's squares and
        # matmuls behind B's weight loads (and vice versa) ----
        with tc.tile_pool(name="psA", bufs=2, space="PSUM") as psA, \
             tc.tile_pool(name="psY", bufs=3, space="PSUM") as psY, \
             tc.tile_pool(name="zwork", bufs=6) as zw, \
             tc.tile_pool(name="ypool", bufs=4) as yp:

            def emit_kT(blk, ch):
                p = psA.tile([128, 512], FP32, tag="pA")
                for ko in range(6):
                    nc.tensor.matmul(p[:], lhsT=wk_sb[:, ko, ts(blk, 128)],
                                     rhs=sT_sb[:, ko, ts(ch, 512)],
                                     start=(ko == 0), stop=(ko == 5))
                nc.vector.tensor_copy(kT_sb[:, blk, ts(ch, 512)], p[:])

            def emit_v(tb, ch):
                p = psA.tile([128, 512], FP32, tag="pA")
                for ko in range(6):
                    nc.tensor.matmul(p[:], lhsT=sT_sb[:, ko, ts(tb, 128)],
                                     rhs=wv_sb[:, ko, ts(ch, 512)],
                                     start=(ko == 0), stop=(ko == 5))
                nc.vector.tensor_copy(
                    v_sb[:, tb, ds(8 * ch, 8), 0:HDP],
                    p.rearrange("p (a b) -> p a b", a=8))

            def emit_q(blk):
                p = psA.tile([128, 512], FP32, tag="pA", name="pQ")[:, :RPC]
                for ko in range(6):
                    nc.tensor.matmul(p[:], lhsT=wq_sb[:, ko, ts(blk, 128)],
                                     rhs=sTc_sb[:, ko, :],
                                     start=(ko == 0), stop=(ko == 5))
                nc.scalar.activation(qT_sb[:, blk, :], p[:], AF.Identity,
                                     bias=bq_sb[:, blk:blk + 1], scale=ISQ)

            def emit_g(ch):
                w = 512 if ch == 0 else 256
                p = psA.tile([128, 512], FP32, tag="pA")
                for ko in range(6):
                    nc.tensor.matmul(p[:, :w], lhsT=sTc_sb[:, ko, :],
                                     rhs=wg_sb[:, ko, ds(512 * ch, w)],
                                     start=(ko == 0), stop=(ko == 5))
                nc.vector.tensor_copy(g_sb[:, ds(512 * ch, w)], p[:, :w])

            def _cap(f, *a):
                return lambda: f(*a)

            b_units = (
                [_cap(emit_kT, blk, ch) for blk in range(8) for ch in range(2)]
                + [_cap(emit_v, tb, ch) for tb in range(8) for ch in range(2)]
                + [_cap(emit_q, blk) for blk in range(8)]
                + [_cap(emit_g, ch) for ch in range(2)]
            )

            def emit_wgo():
                nc.sync.dma_start(wg_sb[:],
                                  WgT.rearrange("(a p) n -> p a n", p=128))
                nc.sync.dma_start(wo_sb[:],
                                  WoT.rearrange("(a p) n -> p a n", p=128))

            # ---- stage C: pair-bias pipeline over own z rows ----
            # z alternates between the HWDGE (sync) and SWDGE (gpsimd) rings;
            # host layout makes each group load fully contiguous per partition
            NB = len(b_units)
            NG = RPC // 4
            done = 0
            for grp in range(NG):
                zT4 = zw.tile([128, 4, S], FP8, tag="zT4")
                # all z on the SWDGE ring (its queue drains much faster);
                # y/stats round-trip traffic moves to sync in exchange
                nc.gpsimd.dma_start(zT4[:], zb[grp])
                if grp == 0:
                    emit_wv()
                sq4 = zw.tile([128, 4, S], FP8, tag="sq4")
                nc.vector.tensor_tensor(sq4[:, 0], zT4[:, 0], zT4[:, 0],
                                        ALU.mult)
                nc.vector.tensor_tensor(sq4[:, 1, 0:512], zT4[:, 1, 0:512],
                                        zT4[:, 1, 0:512], ALU.mult)
                nc.scalar.activation(sq4[:, 1, 512:1024], zT4[:, 1, 512:1024],
                                     AF.Square)
                nc.scalar.activation(sq4[:, 2], zT4[:, 2], AF.Square)
                nc.scalar.activation(sq4[:, 3], zT4[:, 3], AF.Square)
                y4 = yp.tile([128, 2, 512], BF16, tag="y4")
                for a in range(2):
                    # both rows of the bank go through full-width [128, 512]
                    # accumulating matmuls (DR at partition offset 64 trips a
                    # walrus ISA check); the row-odd weights are zero in
                    # partitions 0-63 and vice versa, so accumulation places
                    # each row's outputs without tile_position
                    pa = psY.tile([128, 512], FP32, tag=f"py{a}",
                                  name=f"py{a}")
                    prev = None
                    for u in range(2):
                        j = 2 * a + u
                        rz = zT4[:, j].rearrange("c (i t) -> c i t", i=2)
                        rs = sq4[:, j].rearrange("c (i t) -> c i t", i=2)
                        m1 = nc.tensor.matmul(pa[:], lhsT=wy_sb[:, u], rhs=rz,
                                              start=(u == 0), stop=False,
                                              perf_mode=DR)
                        if prev is not None:
                            tile.add_dep_helper(m1.ins, prev.ins, sync=False,
                                                reason="bank has_written order")
                        prev = nc.tensor.matmul(pa[:], lhsT=ws_sb[:, u], rhs=rs,
                                                start=False, stop=(u == 1),
                                                perf_mode=DR)
                    nc.vector.tensor_copy(y4[:, a, :], pa[:])
                # y write + stats prefetch on sync
                nc.sync.dma_start(
                    y_dram[ds(4 * grp, 4)].rearrange(
                        "(a rr) hi o j -> (rr hi o) a j", a=2),
                    y4[:])
                nc.sync.dma_start(
                    stats_sb[ds(4 * grp, 4)],
                    y_dram[ds(4 * grp, 4), :, 16:18, :])
                # interleave a slice of stage B's projection units
                if grp == 10:
                    emit_wgo()
                upto = NB * (grp + 1) // NG
                while done < upto:
                    b_units[done]()
                    done += 1

        # ---- stage D: r from prefetched stats (scale cancels in rsqrt) ----
        mu_f = stats_sb[:, :, 0, :]
        ez_f = stats_sb[:, :, 1, :]
        var3 = var_sb.rearrange("p (a j) -> p a j", a=2)
        nc.vector.tensor_tensor(var3, mu_f, mu_f, ALU.mult)
        nc.vector.tensor_tensor(var3, ez_f, var3, ALU.subtract)
        lnv = consts.tile([128, S], BF16, name="lnv")
        nc.scalar.activation(lnv[:], var_sb[:], AF.Ln, bias=eps_sb[:])
        nc.scalar.activation(r_sb[:], lnv[:], AF.Exp, scale=-0.5)

        # ---- stage E: attention per head ----
        with tc.tile_pool(name="psE", bufs=2, space="PSUM") as psE, \
             tc.tile_pool(name="head", bufs=2) as hw_pool:
            for h in range(H):
                po2, blk = 64 * (h % 2), h // 2
                y_h = hw_pool.tile([128, 2, 512], BF16, tag="yh")
                eng = nc.sync if h % 2 == 0 else nc.scalar
                eng.dma_start(y_h[:], y_dram[:, :, h, :])
                yf = y_h.rearrange("p a j -> p (a j)")
                t1 = hw_pool.tile([128, S], BF16, tag="t1")
                nc.vector.tensor_tensor(t1[:], yf, r_sb[:], ALU.mult)
                sc = hw_pool.tile([128, S], BF16, tag="sc")
                for ch in range(2):
                    pq = psE.tile([128, 512], FP32, tag="qk")
                    nc.tensor.matmul(pq[:],
                                     lhsT=qT_sb[ds(po2, 64), blk, :],
                                     rhs=kT_sb[ds(po2, 64), blk, ts(ch, 512)],
                                     start=True, stop=True)
                    nc.vector.tensor_tensor(sc[:, ts(ch, 512)], pq[:],
                                            t1[:, ts(ch, 512)], ALU.add)
                aT = hw_pool.tile([128, 8, 128], BF16, tag="aT")
                aTf = aT.rearrange("p a b -> p (a b)")
                for half in range(2):
                    pt = psE.tile([128, 512], BF16, tag="pt")
                    for jj in range(4):
                        nc.tensor.transpose(pt[:, ts(jj, 128)],
                                            sc[:, ts(4 * half + jj, 128)],
                                            ident[:])
                    nc.scalar.activation(aTf[:, ds(512 * half, 512)], pt[:],
                                         AF.Exp)
                po = psE.tile([128, HDP + 1], FP32, tag="po")
                for tb in range(8):
                    nc.tensor.matmul(po[:], lhsT=aT[:, tb, :],
                                     rhs=v_sb[:, tb, h, :],
                                     start=(tb == 0), stop=(tb == 7))
                dr = hw_pool.tile([128, 1], FP32, tag="dr")
                nc.vector.reciprocal(dr[:], po[:, HDP:HDP + 1])
                nc.vector.tensor_scalar(oall[:, ds(HD * h, HD)], po[:, 0:HD],
                                        dr[:], None, op0=ALU.mult)

            # ---- stage F: gate + output projection ----
            sig = hw_pool.tile([128, D], BF16, tag="sig")
            nc.scalar.activation(sig[:], g_sb[:], AF.Sigmoid)
            og = hw_pool.tile([128, D], BF16, tag="og")
            nc.vector.tensor_tensor(og[:], oall[:], sig[:], ALU.mult)
            ogT = hw_pool.tile([128, 6, 128], BF16, tag="ogT")
            for half, n in ((0, 4), (1, 2)):
                pt = psE.tile([128, 512], BF16, tag="pt")
                for jj in range(n):
                    nc.tensor.transpose(pt[:, ts(jj, 128)],
                                        og[:, ts(4 * half + jj, 128)], ident[:])
                nc.vector.tensor_copy(
                    ogT[:, ds(4 * half, n), :],
                    pt.rearrange("p (a b) -> p a b", a=4)[:, 0:n, :])
            out_sb = hw_pool.tile([128, D], FP32, tag="outsb")
            for ch, w in ((0, 512), (1, 256)):
                pf = psE.tile([128, 512], FP32, tag="qk")
                for ko in range(6):
                    nc.tensor.matmul(pf[:, :w], lhsT=ogT[:, ko, :],
                                     rhs=wo_sb[:, ko, ds(512 * ch, w)],
                                     start=(ko == 0), stop=(ko == 5))
                nc.vector.tensor_copy(out_sb[:, ds(512 * ch, w)], pf[:, :w])
            nc.sync.dma_start(out[:], out_sb[:])

    nc.compile()
    return nc


def _prep(inputs):
    bf = ml_dtypes.bfloat16
    f8 = ml_dtypes.float8_e4m3
    s = np.asarray(inputs["s"], np.float32)[0]
    z = np.asarray(inputs["z"], np.float32)[0]
    Wq = np.asarray(inputs["Wq"], np.float32)
    bq = np.asarray(inputs["bq"], np.float32)
    Wk = np.asarray(inputs["Wk"], np.float32)
    Wv = np.asarray(inputs["Wv"], np.float32)
    Wg = np.asarray(inputs["Wg"], np.float32)
    ln_w = np.asarray(inputs["ln_w"], np.float32)
    ln_b = np.asarray(inputs["ln_b"], np.float32)  # noqa: F841 (softmax-invariant)
    Wz = np.asarray(inputs["Wz"], np.float32)
    Wo = np.asarray(inputs["Wo"], np.float32)

    def pad_rows(W):
        Wp = np.zeros((DP, D), np.float32)
        for h in range(H):
            Wp[h * HDP:h * HDP + HD] = W[h * HD:(h + 1) * HD]
        return Wp

    sT = np.ascontiguousarray(s.T).astype(bf)
    WqTp = np.ascontiguousarray(pad_rows(Wq).T).astype(bf)
    WkTp = np.ascontiguousarray(pad_rows(Wk).T).astype(bf)
    WvTp = np.ascontiguousarray(pad_rows(Wv).T).astype(bf)
    WgT = np.ascontiguousarray(Wg.T).astype(bf)
    WoT = np.ascontiguousarray(Wo.T).astype(bf)
    bq_p = np.zeros(DP, np.float32)
    for h in range(H):
        bq_p[h * HDP:h * HDP + HD] = bq[h * HD:(h + 1) * HD]
    bq_p *= ISQ

    # pair-bias weights: mean correction folded in, x32 fp8 pre-scale.
    # Wy[u] places row-u-of-bank outputs at partitions 64u + {32hi + o}.
    Wzp = ln_w[None, :] * Wz                     # [H, DZ]
    c1 = Wzp.sum(-1)                             # [H]
    Wp8 = Wzp.T - c1[None, :] / DZ               # [DZ, H]
    Wy = np.zeros((2, DZ, 2, 128), np.float32)
    Ws = np.zeros((2, DZ, 2, 128), np.float32)
    for u in range(2):
        for hi in range(2):
            base = 64 * u + 32 * hi
            Wy[u, :, hi, base:base + H] = ZSC * Wp8
            Wy[u, :, hi, base + 16] = ZSC / DZ
            Ws[u, :, hi, base + 17] = ZSC * ZSC / DZ

    shared = {
        "sT": sT, "WqT": WqTp, "WkT": WkTp, "WvT": WvTp, "WgT": WgT,
        "WoT": WoT, "bqs": bq_p, "Wy8": Wy.astype(f8), "Ws8": Ws.astype(f8),
    }
    in_maps = []
    for ci in range(N_CORES):
        rows = slice(ci * RPC, (ci + 1) * RPC)
        m = dict(shared)
        m["zb"] = np.ascontiguousarray(
            z[rows].reshape(RPC // 4, 4, S, DZ).transpose(0, 3, 1, 2)
        ).astype(f8)
        m["sTc"] = np.ascontiguousarray(sT[:, rows])
        in_maps.append(m)
    return in_maps


def _install_ntff_hook():
    try:
        import antenv
        from trn_agent_boot.trn_boot import _ntff_profile_via_ctypes
        from concourse import bass_utils
        mod = types.ModuleType("antenv.axon_hooks")
        mod._hook = _ntff_profile_via_ctypes('/opt/axon/libaxon_pjrt.so')
        mod.set_axon_ntff_profile_hook = lambda h: setattr(mod, "_hook", h)
        mod.get_axon_ntff_profile_hook = lambda: mod._hook
        sys.modules["antenv.axon_hooks"] = mod
        antenv.axon_hooks = mod
        bass_utils.upload_artifacts = lambda tmpdir: tmpdir
    except Exception as e:  # profiling is best-effort
        print(f"ntff hook install failed: {e}", file=sys.stderr)


def run(inputs, trace=False):
    from concourse.bass_utils import run_bass_kernel_spmd
    in_maps = _prep(inputs)
    if "nc" not in _CACHE:
        _CACHE["nc"] = _build()
    nc = _CACHE["nc"]
    if trace:
        _install_ntff_hook()
    res = run_bass_kernel_spmd(nc, in_maps, core_ids=list(range(N_CORES)),
                               trace=trace)
    out = np.concatenate([res.results[i]["out"] for i in range(N_CORES)], axis=0)
    return out[None].astype(np.float32), res


def kernel(**inputs) -> np.ndarray:
    out, _ = run(inputs, trace=bool(os.environ.get("KERNEL_TRACE")))
    return out


# revision 70
# speedup vs baseline: 1.0834x; 1.0341x over previous
"""AttentionPairBias kernel for 8 Trainium2 NeuronCores.

Sharding: rows of the query sequence (S=1024) are split across the 8 cores
(128 rows each). The pair tensor z's bias contribution, the softmax and the
output rows are all embarrassingly parallel in the query dimension, so no
collectives are needed; each core reads its own 128x1024x128 slice of z.

Per-core pipeline:
  1. z arrives from the host pre-transposed ([c, t] per query row) and cast
     to fp8e4m3, so group loads are plain contiguous DMAs.
  2. The pair-bias linear runs as fp8 DoubleRow matmuls (K=256): the two
     k-tiles are the two t-halves of a row's zT tile, separated by
     block-diagonal 64-wide weights, so each 512-col stream covers all
     1024 t positions. Outputs pack 2 query rows per PSUM bank at tile
     positions (0,0)/(0,64). Weight columns also carry the LayerNorm mean
     (w=32/128) and, in a second accumulating pass over z^2, E[z^2] (w=8),
     with a global x32 scale that cancels inside rsqrt. The per-head mean
     correction c1[h]/128 is folded into the weights, so bias = r * y
     directly. The ln_b term is dropped (constant along t, softmax-
     invariant).
  3. y rows round-trip through DRAM to re-slice [head-per-partition] into
     [row-per-partition] tiles; stats (mu, E[z^2]) are prefetched back
     per-group so the variance math never waits on a bulk readback.
  4. Per head: scores = qk/sqrt(hd) + r*y_h -> PE transpose -> exp on ACT
     (max-subtraction-free: |scores| < 4) -> A@[V|1] gives o and the
     softmax denominator in one accumulation chain.
  5. sigmoid gate, output projection.
"""

import os
import sys
import types
import numpy as np

for _p in ("/opt/trn_rl_repo", "/root/.axon_site/_ro/trn_rl_repo"):
    if os.path.isdir(_p) and _p not in sys.path:
        sys.path.append(_p)

import ml_dtypes
from contextlib import ExitStack

import concourse.bass as bass
import concourse.mybir as mybir
import concourse.tile as tile
from concourse import bacc
from concourse.bass import ds, ts
from concourse.masks import make_identity

BF16 = mybir.dt.bfloat16
FP32 = mybir.dt.float32
FP8 = mybir.dt.float8e4
AF = mybir.ActivationFunctionType
ALU = mybir.AluOpType
DR = mybir.MatmulPerfMode.DoubleRow

S = 1024
D = 768
H = 16
HD = 48
HDP = 64            # padded head dim (2 heads per 128-partition block)
DP = H * HDP        # 1024
DZ = 128
EPS = 1e-5
N_CORES = 8
RPC = S // N_CORES  # 128 rows per core
ISQ = float(HD) ** -0.5
ZSC = 32.0          # fp8 weight pre-scale; cancels inside rsqrt

_CACHE = {}


def _build():
    nc = bacc.Bacc("TRN2", target_bir_lowering=False, debug=False,
                   num_devices=N_CORES)

    zb = nc.dram_tensor("zb", [RPC // 4, DZ, 4, S], FP8,
                        kind="ExternalInput").ap()
    sT = nc.dram_tensor("sT", [D, S], BF16, kind="ExternalInput").ap()
    sTc = nc.dram_tensor("sTc", [D, RPC], BF16, kind="ExternalInput").ap()
    WqT = nc.dram_tensor("WqT", [D, DP], BF16, kind="ExternalInput").ap()
    WkT = nc.dram_tensor("WkT", [D, DP], BF16, kind="ExternalInput").ap()
    WvT = nc.dram_tensor("WvT", [D, DP], BF16, kind="ExternalInput").ap()
    WgT = nc.dram_tensor("WgT", [D, D], BF16, kind="ExternalInput").ap()
    WoT = nc.dram_tensor("WoT", [D, D], BF16, kind="ExternalInput").ap()
    bqs = nc.dram_tensor("bqs", [DP], FP32, kind="ExternalInput").ap()
    Wy8 = nc.dram_tensor("Wy8", [2, DZ, 2, 128], FP8, kind="ExternalInput").ap()
    Ws8 = nc.dram_tensor("Ws8", [2, DZ, 2, 128], FP8, kind="ExternalInput").ap()
    out = nc.dram_tensor("out", [RPC, D], FP32, kind="ExternalOutput").ap()

    with tile.TileContext(nc) as tc, ExitStack() as ctx:
        consts = ctx.enter_context(tc.tile_pool(name="consts", bufs=1))
        dram = ctx.enter_context(tc.tile_pool(name="dram", bufs=1, space="DRAM"))

        # sT on sync, wk on scalar: the two 1.5MB tensors feeding the first
        # PE matmuls load in parallel (~135 GB/s per queue). Small loads go
        # first on scalar: the framework's conservative startup semaphore
        # makes early matmuls wait on whole dma batches.
        # sT leads the sync queue, followed by the tiny pair-bias weights, so
        # the first batch of sync dmas (coarse semaphore granularity)
        # completes as soon as sT lands
        sT_sb = consts.tile([128, 6, S], BF16, name="sT_sb")
        nc.sync.dma_start(sT_sb[:], sT.rearrange("(a p) n -> p a n", p=128))
        wy_sb = consts.tile([128, 2, 2, 128], FP8, name="wy_sb")
        nc.sync.dma_start(wy_sb[:], Wy8.rearrange("r c i m -> c r i m"))
        ws_sb = consts.tile([128, 2, 2, 128], FP8, name="ws_sb")
        nc.sync.dma_start(ws_sb[:], Ws8.rearrange("r c i m -> c r i m"))
        bq_sb = consts.tile([128, 8], FP32, name="bq_sb")
        nc.sync.dma_start(bq_sb[:], bqs.rearrange("(b p) -> p b", p=128))
        wk_sb = consts.tile([128, 6, DP], BF16, name="wk_sb")
        nc.scalar.dma_start(wk_sb[:], WkT.rearrange("(a p) n -> p a n", p=128))
        sTc_sb = consts.tile([128, 6, RPC], BF16, name="sTc_sb")
        nc.scalar.dma_start(sTc_sb[:], sTc.rearrange("(a p) n -> p a n", p=128))
        wq_sb = consts.tile([128, 6, DP], BF16, name="wq_sb")
        nc.sync.dma_start(wq_sb[:], WqT.rearrange("(a p) n -> p a n", p=128))
        wg_sb = consts.tile([128, 6, D], BF16, name="wg_sb")
        wo_sb = consts.tile([128, 6, D], BF16, name="wo_sb")
        # wv on scalar (v-projections start mid-loop); the SWDGE ring carries
        # only the z stream
        wv_sb = consts.tile([128, 6, DP], BF16, name="wv_sb")
        nc.scalar.dma_start(wv_sb[:], WvT.rearrange("(a p) n -> p a n", p=128))
        ident = consts.tile([128, 128], BF16, name="ident")
        make_identity(nc, ident[:])
        eps_sb = consts.tile([128, 1], FP32, name="eps_sb")
        nc.vector.memset(eps_sb[:], EPS * ZSC * ZSC)

        kT_sb = consts.tile([128, 8, S], BF16, name="kT_sb")
        v_sb = consts.tile([128, 8, H, HDP + 1], BF16, name="v_sb")
        qT_sb = consts.tile([128, 8, RPC], BF16, name="qT_sb")
        g_sb = consts.tile([128, D], BF16, name="g_sb")
        oall = consts.tile([128, D], BF16, name="oall")
        # stats: [s, hi, {mu, ez2}, j] prefetched per-group from y_dram
        stats_sb = consts.tile([128, 2, 2, 512], BF16, name="stats_sb")
        r_sb = consts.tile([128, S], BF16, name="r_sb")
        var_sb = consts.tile([128, S], BF16, name="var_sb")

        # y round-trip buffer, query-row-linear: [s, hi, out, j]; the group
        # write AP permutes (bank, row-in-bank) into place
        y_dram = dram.tile([RPC, 2, 32, 512], BF16)

        nc.vector.memset(v_sb[:, :, :, HDP:HDP + 1], 1.0)

        # ---- stage B (projections) + stage C (pair-bias) share pools and
        # are INTERLEAVED in program order: engine streams execute in-order,
        # so emitting all of B first would head-block stage C's squares and
        # matmuls behind B's weight loads (and vice versa) ----
        with tc.tile_pool(name="psA", bufs=2, space="PSUM") as psA, \
             tc.tile_pool(name="psY", bufs=3, space="PSUM") as psY, \
             tc.tile_pool(name="zwork", bufs=6) as zw, \
             tc.tile_pool(name="ypool", bufs=4) as yp:

            def emit_kT(blk, ch):
                p = psA.tile([128, 512], FP32, tag="pA")
                for ko in range(6):
                    nc.tensor.matmul(p[:], lhsT=wk_sb[:, ko, ts(blk, 128)],
                                     rhs=sT_sb[:, ko, ts(ch, 512)],
                                     start=(ko == 0), stop=(ko == 5))
                nc.vector.tensor_copy(kT_sb[:, blk, ts(ch, 512)], p[:])

            def emit_v(tb, ch):
                p = psA.tile([128, 512], FP32, tag="pA")
                for ko in range(6):
                    nc.tensor.matmul(p[:], lhsT=sT_sb[:, ko, ts(tb, 128)],
                                     rhs=wv_sb[:, ko, ts(ch, 512)],
                                     start=(ko == 0), stop=(ko == 5))
                nc.vector.tensor_copy(
                    v_sb[:, tb, ds(8 * ch, 8), 0:HDP],
                    p.rearrange("p (a b) -> p a b", a=8))

            def emit_q(blk):
                p = psA.tile([128, 512], FP32, tag="pA", name="pQ")[:, :RPC]
                for ko in range(6):
                    nc.tensor.matmul(p[:], lhsT=wq_sb[:, ko, ts(blk, 128)],
                                     rhs=sTc_sb[:, ko, :],
                                     start=(ko == 0), stop=(ko == 5))
                nc.scalar.activation(qT_sb[:, blk, :], p[:], AF.Identity,
                                     bias=bq_sb[:, blk:blk + 1], scale=ISQ)

            def emit_g(ch):
                w = 512 if ch == 0 else 256
                p = psA.tile([128, 512], FP32, tag="pA")
                for ko in range(6):
                    nc.tensor.matmul(p[:, :w], lhsT=sTc_sb[:, ko, :],
                                     rhs=wg_sb[:, ko, ds(512 * ch, w)],
                                     start=(ko == 0), stop=(ko == 5))
                nc.vector.tensor_copy(g_sb[:, ds(512 * ch, w)], p[:, :w])

            def _cap(f, *a):
                return lambda: f(*a)

            b_units = (
                [_cap(emit_kT, blk, ch) for blk in range(8) for ch in range(2)]
                + [_cap(emit_v, tb, ch) for tb in range(8) for ch in range(2)]
                + [_cap(emit_q, blk) for blk in range(8)]
                + [_cap(emit_g, ch) for ch in range(2)]
            )

            def emit_wgo():
                nc.sync.dma_start(wg_sb[:],
                                  WgT.rearrange("(a p) n -> p a n", p=128))
                nc.sync.dma_start(wo_sb[:],
                                  WoT.rearrange("(a p) n -> p a n", p=128))

            # ---- stage C: pair-bias pipeline over own z rows ----
            # z alternates between the HWDGE (sync) and SWDGE (gpsimd) rings;
            # host layout makes each group load fully contiguous per partition
            NB = len(b_units)
            NG = RPC // 4
            done = 0
            for grp in range(NG):
                zT4 = zw.tile([128, 4, S], FP8, tag="zT4")
                # all z on the SWDGE ring (its queue drains much faster);
                # y/stats round-trip traffic moves to sync in exchange
                nc.gpsimd.dma_start(zT4[:], zb[grp])
                sq4 = zw.tile([128, 4, S], FP8, tag="sq4")
                nc.vector.tensor_tensor(sq4[:, 0], zT4[:, 0], zT4[:, 0],
                                        ALU.mult)
                nc.vector.tensor_tensor(sq4[:, 1, 0:512], zT4[:, 1, 0:512],
                                        zT4[:, 1, 0:512], ALU.mult)
                nc.scalar.activation(sq4[:, 1, 512:1024], zT4[:, 1, 512:1024],
                                     AF.Square)
                nc.scalar.activation(sq4[:, 2], zT4[:, 2], AF.Square)
                nc.scalar.activation(sq4[:, 3], zT4[:, 3], AF.Square)
                y4 = yp.tile([128, 2, 512], BF16, tag="y4")
                for a in range(2):
                    # both rows of the bank go through full-width [128, 512]
                    # accumulating matmuls (DR at partition offset 64 trips a
                    # walrus ISA check); the row-odd weights are zero in
                    # partitions 0-63 and vice versa, so accumulation places
                    # each row's outputs without tile_position
                    pa = psY.tile([128, 512], FP32, tag=f"py{a}",
                                  name=f"py{a}")
                    prev = None
                    for u in range(2):
                        j = 2 * a + u
                        rz = zT4[:, j].rearrange("c (i t) -> c i t", i=2)
                        rs = sq4[:, j].rearrange("c (i t) -> c i t", i=2)
                        m1 = nc.tensor.matmul(pa[:], lhsT=wy_sb[:, u], rhs=rz,
                                              start=(u == 0), stop=False,
                                              perf_mode=DR)
                        if prev is not None:
                            tile.add_dep_helper(m1.ins, prev.ins, sync=False,
                                                reason="bank has_written order")
                        prev = nc.tensor.matmul(pa[:], lhsT=ws_sb[:, u], rhs=rs,
                                                start=False, stop=(u == 1),
                                                perf_mode=DR)
                    nc.vector.tensor_copy(y4[:, a, :], pa[:])
                # y write + stats prefetch on sync
                nc.sync.dma_start(
                    y_dram[ds(4 * grp, 4)].rearrange(
                        "(a rr) hi o j -> (rr hi o) a j", a=2),
                    y4[:])
                nc.sync.dma_start(
                    stats_sb[ds(4 * grp, 4)],
                    y_dram[ds(4 * grp, 4), :, 16:18, :])
                # interleave a slice of stage B's projection units
                if grp == 10:
                    emit_wgo()
                upto = NB * (grp + 1) // NG
                while done < upto:
                    b_units[done]()
                    done += 1

        # ---- stage D: r from prefetched stats (scale cancels in rsqrt) ----
        mu_f = stats_sb[:, :, 0, :]
        ez_f = stats_sb[:, :, 1, :]
        var3 = var_sb.rearrange("p (a j) -> p a j", a=2)
        nc.vector.tensor_tensor(var3, mu_f, mu_f, ALU.mult)
        nc.vector.tensor_tensor(var3, ez_f, var3, ALU.subtract)
        lnv = consts.tile([128, S], BF16, name="lnv")
        nc.scalar.activation(lnv[:], var_sb[:], AF.Ln, bias=eps_sb[:])
        nc.scalar.activation(r_sb[:], lnv[:], AF.Exp, scale=-0.5)

        # ---- stage E: attention per head ----
        with tc.tile_pool(name="psE", bufs=2, space="PSUM") as psE, \
             tc.tile_pool(name="head", bufs=2) as hw_pool:
            for h in range(H):
                po2, blk = 64 * (h % 2), h // 2
                y_h = hw_pool.tile([128, 2, 512], BF16, tag="yh")
                eng = nc.sync if h % 2 == 0 else nc.scalar
                eng.dma_start(y_h[:], y_dram[:, :, h, :])
                yf = y_h.rearrange("p a j -> p (a j)")
                t1 = hw_pool.tile([128, S], BF16, tag="t1")
                nc.vector.tensor_tensor(t1[:], yf, r_sb[:], ALU.mult)
                sc = hw_pool.tile([128, S], BF16, tag="sc")
                for ch in range(2):
                    pq = psE.tile([128, 512], FP32, tag="qk")
                    nc.tensor.matmul(pq[:],
                                     lhsT=qT_sb[ds(po2, 64), blk, :],
                                     rhs=kT_sb[ds(po2, 64), blk, ts(ch, 512)],
                                     start=True, stop=True)
                    nc.vector.tensor_tensor(sc[:, ts(ch, 512)], pq[:],
                                            t1[:, ts(ch, 512)], ALU.add)
                aT = hw_pool.tile([128, 8, 128], BF16, tag="aT")
                aTf = aT.rearrange("p a b -> p (a b)")
                for half in range(2):
                    pt = psE.tile([128, 512], BF16, tag="pt")
                    for jj in range(4):
                        nc.tensor.transpose(pt[:, ts(jj, 128)],
                                            sc[:, ts(4 * half + jj, 128)],
                                            ident[:])
                    nc.scalar.activation(aTf[:, ds(512 * half, 512)], pt[:],
                                         AF.Exp)
                po = psE.tile([128, HDP + 1], FP32, tag="po")
                for tb in range(8):
                    nc.tensor.matmul(po[:], lhsT=aT[:, tb, :],
                                     rhs=v_sb[:, tb, h, :],
                                     start=(tb == 0), stop=(tb == 7))
                dr = hw_pool.tile([128, 1], FP32, tag="dr")
                nc.vector.reciprocal(dr[:], po[:, HDP:HDP + 1])
                nc.vector.tensor_scalar(oall[:, ds(HD * h, HD)], po[:, 0:HD],
                                        dr[:], None, op0=ALU.mult)

            # ---- stage F: gate + output projection ----
            sig = hw_pool.tile([128, D], BF16, tag="sig")
            nc.scalar.activation(sig[:], g_sb[:], AF.Sigmoid)
            og = hw_pool.tile([128, D], BF16, tag="og")
            nc.vector.tensor_tensor(og[:], oall[:], sig[:], ALU.mult)
            ogT = hw_pool.tile([128, 6, 128], BF16, tag="ogT")
            for half, n in ((0, 4), (1, 2)):
                pt = psE.tile([128, 512], BF16, tag="pt")
                for jj in range(n):
                    nc.tensor.transpose(pt[:, ts(jj, 128)],
                                        og[:, ts(4 * half + jj, 128)], ident[:])
                nc.vector.tensor_copy(
                    ogT[:, ds(4 * half, n), :],
                    pt.rearrange("p (a b) -> p a b", a=4)[:, 0:n, :])
            out_sb = hw_pool.tile([128, D], FP32, tag="outsb")
            for ch, w in ((0, 512), (1, 256)):
                pf = psE.tile([128, 512], FP32, tag="qk")
                for ko in range(6):
                    nc.tensor.matmul(pf[:, :w], lhsT=ogT[:, ko, :],
                                     rhs=wo_sb[:, ko, ds(512 * ch, w)],
                                     start=(ko == 0), stop=(ko == 5))
                nc.vector.tensor_copy(out_sb[:, ds(512 * ch, w)], pf[:, :w])
            nc.sync.dma_start(out[:], out_sb[:])

    nc.compile()
    return nc


def _prep(inputs):
    bf = ml_dtypes.bfloat16
    f8 = ml_dtypes.float8_e4m3
    s = np.asarray(inputs["s"], np.float32)[0]
    z = np.asarray(inputs["z"], np.float32)[0]
    Wq = np.asarray(inputs["Wq"], np.float32)
    bq = np.asarray(inputs["bq"], np.float32)
    Wk = np.asarray(inputs["Wk"], np.float32)
    Wv = np.asarray(inputs["Wv"], np.float32)
    Wg = np.asarray(inputs["Wg"], np.float32)
    ln_w = np.asarray(inputs["ln_w"], np.float32)
    ln_b = np.asarray(inputs["ln_b"], np.float32)  # noqa: F841 (softmax-invariant)
    Wz = np.asarray(inputs["Wz"], np.float32)
    Wo = np.asarray(inputs["Wo"], np.float32)

    def pad_rows(W):
        Wp = np.zeros((DP, D), np.float32)
        for h in range(H):
            Wp[h * HDP:h * HDP + HD] = W[h * HD:(h + 1) * HD]
        return Wp

    sT = np.ascontiguousarray(s.T).astype(bf)
    WqTp = np.ascontiguousarray(pad_rows(Wq).T).astype(bf)
    WkTp = np.ascontiguousarray(pad_rows(Wk).T).astype(bf)
    WvTp = np.ascontiguousarray(pad_rows(Wv).T).astype(bf)
    WgT = np.ascontiguousarray(Wg.T).astype(bf)
    WoT = np.ascontiguousarray(Wo.T).astype(bf)
    bq_p = np.zeros(DP, np.float32)
    for h in range(H):
        bq_p[h * HDP:h * HDP + HD] = bq[h * HD:(h + 1) * HD]
    bq_p *= ISQ

    # pair-bias weights: mean correction folded in, x32 fp8 pre-scale.
    # Wy[u] places row-u-of-bank outputs at partitions 64u + {32hi + o}.
    Wzp = ln_w[None, :] * Wz                     # [H, DZ]
    c1 = Wzp.sum(-1)                             # [H]
    Wp8 = Wzp.T - c1[None, :] / DZ               # [DZ, H]
    Wy = np.zeros((2, DZ, 2, 128), np.float32)
    Ws = np.zeros((2, DZ, 2, 128), np.float32)
    for u in range(2):
        for hi in range(2):
            base = 64 * u + 32 * hi
            Wy[u, :, hi, base:base + H] = ZSC * Wp8
            Wy[u, :, hi, base + 16] = ZSC / DZ
            Ws[u, :, hi, base + 17] = ZSC * ZSC / DZ

    shared = {
        "sT": sT, "WqT": WqTp, "WkT": WkTp, "WvT": WvTp, "WgT": WgT,
        "WoT": WoT, "bqs": bq_p, "Wy8": Wy.astype(f8), "Ws8": Ws.astype(f8),
    }
    in_maps = []
    for ci in range(N_CORES):
        rows = slice(ci * RPC, (ci + 1) * RPC)
        m = dict(shared)
        m["zb"] = np.ascontiguousarray(
            z[rows].reshape(RPC // 4, 4, S, DZ).transpose(0, 3, 1, 2)
        ).astype(f8)
        m["sTc"] = np.ascontiguousarray(sT[:, rows])
        in_maps.append(m)
    return in_maps


def _install_ntff_hook():
    try:
        import antenv
        from trn_agent_boot.trn_boot import _ntff_profile_via_ctypes
        from concourse import bass_utils
        mod = types.ModuleType("antenv.axon_hooks")
        mod._hook = _ntff_profile_via_ctypes('/opt/axon/libaxon_pjrt.so')
        mod.set_axon_ntff_profile_hook = lambda h: setattr(mod, "_hook", h)
        mod.get_axon_ntff_profile_hook = lambda: mod._hook
        sys.modules["antenv.axon_hooks"] = mod
        antenv.axon_hooks = mod
        bass_utils.upload_artifacts = lambda tmpdir: tmpdir
    except Exception as e:  # profiling is best-effort
        print(f"ntff hook install failed: {e}", file=sys.stderr)


def run(inputs, trace=False):
    from concourse.bass_utils import run_bass_kernel_spmd
    in_maps = _prep(inputs)
    if "nc" not in _CACHE:
        _CACHE["nc"] = _build()
    nc = _CACHE["nc"]
    if trace:
        _install_ntff_hook()
    res = run_bass_kernel_spmd(nc, in_maps, core_ids=list(range(N_CORES)),
                               trace=trace)
    out = np.concatenate([res.results[i]["out"] for i in range(N_CORES)], axis=0)
    return out[None].astype(np.float32), res


def kernel(**inputs) -> np.ndarray:
    out, _ = run(inputs, trace=bool(os.environ.get("KERNEL_TRACE")))
    return out
